# revision 22
# baseline (speedup 1.0000x reference)
"""Trainium2 Bass kernel for nn_CrossChannelAttention.

Reference computation (per batch b, pixel p, with C=128 channels, NUMS=16
groups of HEADS=8 channels, OUT=256):
    fm[g,p]  = relu(sum_h W1[g,h] * x[8g+h, p] + b1[g])          # [16, P]
    feat[(g,d), p] = fm[g,p] * x[d,p]                            # [2048, P]
    out[o,p] = sum_c W2[o,c] * feat[c,p] + b2[o]                 # [256, P]

Data-parallel over batch B=8 across the 8 NeuronCores (one image per core,
params replicated).  Per core:
  - PE floor: 256 accumulating K=128 N=512 bf16 matmuls (~55us at the warm
    2.4 GHz p-state).  Dummy warmup matmuls on a memset scratch tile ramp the
    p-state (3us of continuous PE busy -> 2.4 GHz) while input DMAs run, so
    the real matmuls never execute at the cold 1.2 GHz rate.
  - fm rows are broadcast to 128 partitions in [128,1024] chunks, split
    between DRAM->SBUF broadcast DMAs (13 groups; wide shapes fan out across
    all 16 DMA engines) and gpsimd.partition_broadcast (groups 12-14, read
    from a packed partition-0 SBUF row copied by one tiny DMA per chunk,
    skipping the DRAM round-trip).
  - feat = x * fm_rep on the vector engine as pure-SBUF bf16 multiplies,
    pipelined LOOKAHEAD units ahead of the consuming matmuls.
  - head ordering: all four x chunks plus the split W2 halves are issued
    before any relu-gated fm write, so no input load ever queues behind the
    first broadcast chain (the previous version lost ~12us to this).
  - outputs are written bf16 (host casts to fp32), fused [oc0|oc1] per
    pixel block: half the store traffic and a short tail.
Accuracy: bf16 matmuls with fp32 PSUM accumulation; rel err ~4e-3.
"""

import numpy as np
import ml_dtypes

import concourse.bacc as bacc
import concourse.tile as tile
from concourse import mybir
from concourse.bass_utils import run_bass_kernel_spmd

F32 = mybir.dt.float32
BF16 = mybir.dt.bfloat16

B, C, H, W = 8, 128, 64, 64
NUMS, HEADS, OUT = 16, 8, 256
P = H * W          # 4096 pixels per image
PB = 512           # pixel block (one PSUM bank of fp32)
GRP = 1024         # broadcast chunk (2 pixel blocks)
NGRP = P // GRP    # 4 broadcast groups
N_CORES = 8
LOOKAHEAD = 8      # broadcast/feat pipeline depth (in (g,k) units)
GP_GS = (12, 13, 14)   # groups replicated via gpsimd.partition_broadcast
GP_LO, GP_HI = GP_GS[0], GP_GS[-1] + 1
W2A_G = 6          # groups 0..5 in the first W2 load

NWARM_PRE = 6      # PE p-state ramp matmuls before fm k0
NWARM_MID = 2      # PE filler between fm chunks
NWARM_POST = 7     # PE filler between last fm and first main

_CACHE = {}


def _build():
    nc = bacc.Bacc("TRN2", target_bir_lowering=False, debug=False,
                   num_devices=N_CORES)

    x_d = nc.dram_tensor("x", [C, P], BF16, kind="ExternalInput")
    w1s_d = nc.dram_tensor("w1s", [C, NUMS], BF16, kind="ExternalInput")
    w2t_d = nc.dram_tensor("w2t", [C, NUMS * OUT], BF16, kind="ExternalInput")
    b1_d = nc.dram_tensor("b1c", [NUMS, 1], F32, kind="ExternalInput")
    b2_d = nc.dram_tensor("b2c", [C, 2], F32, kind="ExternalInput")
    # c-major bf16 output: host reorders to [OUT, P] fp32
    out_d = nc.dram_tensor("out", [C, 2, P // PB, PB], BF16,
                           kind="ExternalOutput")

    relu = mybir.ActivationFunctionType.Relu
    ident = mybir.ActivationFunctionType.Identity
    mult = mybir.AluOpType.mult

    with tile.TileContext(nc) as tc:
        with (
            tc.tile_pool(name="const", bufs=1) as cpool,
            tc.tile_pool(name="fmgp", bufs=2) as fmgpp,
            tc.tile_pool(name="repp", bufs=10) as repp,
            tc.tile_pool(name="gprep", bufs=5) as gprepp,
            tc.tile_pool(name="feat", bufs=2 * LOOKAHEAD + 2) as featp,
            tc.tile_pool(name="osb", bufs=3) as osbp,
            tc.tile_pool(name="psw", bufs=1, space="PSUM") as psw,
            tc.tile_pool(name="psfm", bufs=2, space="PSUM") as psfmp,
            tc.tile_pool(name="ps", bufs=5, space="PSUM") as ps,
            tc.tile_pool(name="dr", bufs=1, space="DRAM") as drp,
        ):
            # ---- t=0: memset scratch, preload act table, ramp PE ----
            scratch = cpool.tile([C, PB], BF16)
            nc.vector.memset(scratch[:], 0.0)

            ps_w = psw.tile([C, PB], F32, tag="psw", name="ps_warm")

            def warm(n):
                for _ in range(n):
                    nc.tensor.matmul(ps_w[:], scratch[:, 0:C], scratch[:],
                                     start=True, stop=True)

            warm(NWARM_PRE)

            # scalar queue: w1s/b1 first (fm needs them), then act-table
            # preload, then the split W2 + b2
            w1s_t = cpool.tile([C, NUMS], BF16)
            nc.scalar.dma_start(w1s_t[:], w1s_d[:])
            b1_t = cpool.tile([NUMS, 1], F32)
            nc.scalar.dma_start(b1_t[:], b1_d[:])
            dummy = cpool.tile([NUMS, 1], BF16)
            nc.scalar.activation(dummy[:], scratch[0:NUMS, 0:1], relu)
            w2a = cpool.tile([C, W2A_G * 2 * C], BF16)
            nc.scalar.dma_start(w2a[:], w2t_d[:, 0:W2A_G * 2 * C])
            w2b = cpool.tile([C, (NUMS - W2A_G) * 2 * C], BF16)
            nc.scalar.dma_start(w2b[:], w2t_d[:, W2A_G * 2 * C:])
            b2_t = cpool.tile([C, 2], F32)
            nc.scalar.dma_start(b2_t[:], b2_d[:])

            def w2(g, oc):
                i = 2 * g + oc
                if g < W2A_G:
                    return w2a[:, i * C:(i + 1) * C]
                i -= 2 * W2A_G
                return w2b[:, i * C:(i + 1) * C]

            # sync queue: all four x chunks up front; x0 as two half DMAs
            # so its transfer runs on two DMA ring-groups concurrently
            x2s = []
            for k in range(NGRP):
                x2 = cpool.tile([C, GRP], BF16, tag=f"x2_{k}", name=f"x2_{k}")
                x2s.append(x2)
                gx = slice(k * GRP, (k + 1) * GRP)
                if k == 0:
                    nc.sync.dma_start(x2[:, 0:PB], x_d[:, 0:PB])
                    nc.sync.dma_start(x2[:, PB:GRP], x_d[:, PB:GRP])
                else:
                    nc.sync.dma_start(x2[:], x_d[:, gx])

            # ---- fm: matmul + relu per k chunk, PE fillers between ----
            fm_sb = cpool.tile([NUMS, P], BF16)
            fm_drs = [drp.tile([NUMS, GRP], BF16, tag=f"fmdr{k}",
                               name=f"fmdr{k}") for k in range(NGRP)]
            fm_gps = []

            def emit_pack(k):
                """pack gp rows to partition 0, right after k's relus"""
                gx = slice(k * GRP, (k + 1) * GRP)
                fm_gp = fmgpp.tile([1, len(GP_GS) * GRP], BF16, tag="fmgp",
                                   name=f"fmgp{k}")
                fm_gps.append(fm_gp)
                nc.scalar.dma_start(fm_gp[:], fm_sb[GP_LO:GP_HI, gx])

            for k in range(NGRP):
                for half in range(2):
                    pb = 2 * k + half
                    px = slice(pb * PB, (pb + 1) * PB)
                    hx = slice(half * PB, (half + 1) * PB)
                    ps_fm = psfmp.tile([NUMS, PB], F32, tag="psfm",
                                       name=f"psfm{pb}")
                    nc.tensor.matmul(ps_fm[:], w1s_t[:], x2s[k][:, hx],
                                     start=True, stop=True)
                    nc.scalar.activation(fm_sb[:, px], ps_fm[:], relu,
                                         bias=b1_t[:])
                if k == 0:
                    # k0's fm write + its first broadcasts are issued HERE
                    # on the scalar queue, right behind the relus: no
                    # cross-queue semaphore hop on the critical chain
                    nc.scalar.dma_start(fm_drs[0][:, 0:PB], fm_sb[:, 0:PB])
                    nc.scalar.dma_start(fm_drs[0][:, PB:GRP],
                                        fm_sb[:, PB:GRP])
                emit_pack(k)
                if k < NGRP - 1:
                    warm(NWARM_MID)
            warm(NWARM_POST)

            # ---- fm writes (k>=1, sync queue) ----
            def emit_fm_write(k):
                gx = slice(k * GRP, (k + 1) * GRP)
                nc.sync.dma_start(fm_drs[k][:], fm_sb[:, gx])

            # ---- replication + feat, pipelined ahead of the mains ----
            # k=0: per-group broadcasts (first two groups split in halves
            # for low latency); k>=1: two-group fused broadcasts (one DMA
            # per pair via the flattened fm DRAM buffer) to halve the
            # sequencer issue count
            nbc = [0]
            pair_reps = {}    # (k, glo) -> rep tile [C, 2*GRP]

            def bc_eng():
                eng = nc.sync if nbc[0] % 3 != 2 else nc.scalar
                nbc[0] += 1
                return eng

            def emit_rep_grp(g, k):
                """returns (rep_tile, column offset of g within it)"""
                if g in GP_GS:
                    rep = gprepp.tile([C, GRP], BF16, tag="gprep",
                                      name=f"rep{g}_{k}")
                    src = fm_gps[k][0:1,
                                    (g - GP_LO) * GRP:(g - GP_LO + 1) * GRP]
                    nc.gpsimd.partition_broadcast(rep[:], src)
                    return rep, 0
                if k == 0:
                    rep = repp.tile([C, GRP], BF16, tag="rep",
                                    name=f"rep{g}_{k}")
                    row = fm_drs[0][g:g + 1, :]
                    if g < 2:
                        nc.sync.dma_start(
                            rep[:, 0:PB],
                            row[:, 0:PB].broadcast_to((C, PB)))
                        nc.sync.dma_start(
                            rep[:, PB:GRP],
                            row[:, PB:GRP].broadcast_to((C, PB)))
                    else:
                        bc_eng().dma_start(rep[:],
                                           row.broadcast_to((C, GRP)))
                    return rep, 0
                if g == 15:
                    rep = repp.tile([C, GRP], BF16, tag="rep",
                                    name=f"rep{g}_{k}")
                    bc_eng().dma_start(
                        rep[:], fm_drs[k][g:g + 1, :].broadcast_to((C, GRP)))
                    return rep, 0
                glo = g - (g % 2)
                if (k, glo) not in pair_reps:
                    rep = repp.tile([C, 2 * GRP], BF16, tag="rep2",
                                    name=f"rep{glo}_{k}", bufs=8)
                    src = (fm_drs[k][:]
                           .flatten()
                           .unsqueeze(0)[0:1, glo * GRP:(glo + 2) * GRP]
                           .broadcast_to((C, 2 * GRP)))
                    bc_eng().dma_start(rep[:], src)
                    pair_reps[(k, glo)] = rep
                return pair_reps[(k, glo)], (g - glo) * GRP

            fts = {}      # (g, k) -> [C, GRP] feat tile

            def emit_ft(g, k):
                rep, off = emit_rep_grp(g, k)
                ft = featp.tile([C, GRP], BF16, tag="ft", name=f"ft{g}_{k}")
                nc.vector.tensor_tensor(ft[:], x2s[k][:],
                                        rep[:, off:off + GRP], op=mult)
                fts[(g, k)] = ft

            todo = [(g, k) for k in range(NGRP) for g in range(NUMS)]
            for i in range(LOOKAHEAD):
                emit_ft(*todo[i])

            pso = {}
            for i, (g, k) in enumerate(todo):
                if g == 0 and k + 1 < NGRP:
                    emit_fm_write(k + 1)
                if i + LOOKAHEAD < len(todo):
                    emit_ft(*todo[i + LOOKAHEAD])
                ft = fts.pop((g, k))
                if g == 0:
                    for pbb in (2 * k, 2 * k + 1):
                        for oc in range(2):
                            t = ps.tile([C, PB], F32, tag="ps",
                                        name=f"pso{pbb}_{oc}")
                            pso[(pbb, oc)] = t
                for half in range(2):
                    pb = 2 * k + half
                    hx = slice(half * PB, (half + 1) * PB)
                    for oc in range(2):
                        nc.tensor.matmul(pso[(pb, oc)][:], w2(g, oc),
                                         ft[:, hx], start=(g == 0),
                                         stop=(g == NUMS - 1))
                if g == NUMS - 1:
                    for pbb in (2 * k, 2 * k + 1):
                        last = (pbb == 2 * NGRP - 1)
                        ot = osbp.tile([C, 2, PB], BF16, tag="ot",
                                       name=f"ot{pbb}")
                        for oc in range(2):
                            dst = ot[:, oc:oc + 1, :]
                            src = pso.pop((pbb, oc))[:]
                            if last and oc == 1:
                                # very last block: bias-add on the vector
                                # engine so it runs concurrently with the
                                # scalar act (PE is already done, so the
                                # DVE-reads-PSUM clock penalty is moot)
                                nc.vector.tensor_scalar_add(
                                    dst, src, b2_t[:, 1:2])
                            else:
                                nc.scalar.activation(
                                    dst, src, ident,
                                    bias=b2_t[:, oc:oc + 1])
                        # store halves on separate queues/rings
                        nc.sync.dma_start(out_d[:, 0:1, pbb, :],
                                          ot[:, 0:1, :])
                        nc.scalar.dma_start(out_d[:, 1:2, pbb, :],
                                            ot[:, 1:2, :])

    nc.compile()
    return nc


def _prep_params(W1, b1, W2, b2):
    bf = ml_dtypes.bfloat16
    # w1s[c, g] = W1[g, c - 8g] for 8g <= c < 8(g+1), else 0
    w1s = np.zeros((C, NUMS), dtype=bf)
    for g in range(NUMS):
        w1s[g * HEADS:(g + 1) * HEADS, g] = W1[g].astype(bf)
    # w2t[k, (g*2+oc)*128 + m] = W2[oc*128 + m, g*128 + k]
    w2t = (
        np.asarray(W2, dtype=np.float32)
        .reshape(2, C, NUMS, C)          # [oc, m, g, k]
        .transpose(3, 2, 0, 1)           # [k, g, oc, m]
        .reshape(C, NUMS * OUT)
        .astype(bf)
    )
    b1c = np.asarray(b1, dtype=np.float32).reshape(NUMS, 1).copy()
    b2c = np.asarray(b2, dtype=np.float32).reshape(2, C).T.copy()
    return w1s, w2t, b1c, b2c


def kernel(x, W1, b1, W2, b2, _trace=False, _trace_kwargs=None):
    if "nc" not in _CACHE:
        _CACHE["nc"] = _build()
    nc = _CACHE["nc"]

    w1s, w2t, b1c, b2c = _prep_params(W1, b1, W2, b2)
    xs = np.ascontiguousarray(
        np.asarray(x, dtype=np.float32).reshape(B, C, P).astype(ml_dtypes.bfloat16))
    in_maps = [
        {"x": xs[b_], "w1s": w1s, "w2t": w2t, "b1c": b1c, "b2c": b2c}
        for b_ in range(N_CORES)
    ]
    kwargs = {}
    if _trace:
        kwargs["trace"] = True
        kwargs.update(_trace_kwargs or {})
    res = run_bass_kernel_spmd(nc, in_maps, core_ids=list(range(N_CORES)),
                               **kwargs)
    # out buffer is [C, 2, P//PB, PB] bf16 c-major; reorder to [OUT, P] fp32
    out = np.stack([
        np.asarray(res.results[b_]["out"])
        .transpose(1, 0, 2, 3).reshape(OUT, P)
        for b_ in range(N_CORES)
    ]).astype(np.float32)
    out = out.reshape(B, OUT, H, W)
    if _trace:
        _CACHE["last_result"] = res
    return out
